# revision 9
# baseline (speedup 1.0000x reference)
"""MoE feed-forward (E=8 experts, top-2) on 8 Trainium2 NeuronCores.

Strategy (expert-parallel, per the sharding hint): the router runs on host
in fp32 (bit-matches the reference's routing decisions), tokens are
dispatched to their experts host-side (the "all-to-all"), and each core
runs one expert's full gated-FFN over its gathered token batch with
float32r matmuls (TF32-grade precision at bf16 throughput). The per-token
combine weight is applied on-device; host scatter-adds the two expert
contributions per token and returns (output, aux_loss) like the reference.
"""
import sys

sys.path.insert(0, "/opt/trn_rl_repo")

import numpy as np

H = 1024
E = 8
K = 2
I = 1024
AUX_COEF = 0.01
TB = 384  # token block (moving free dim; >=256 keeps float32r at full rate)

_nc_cache = {}


# ---------------------------------------------------------------------------
# walrus workaround: this build rejects instructions with >1 sync wait
# ---------------------------------------------------------------------------
def _install_tile_fixes():
    import concourse.tile as tile
    import concourse.mybir as mybir
    from concourse.vector_clock import ScopedClock

    if getattr(tile.TileContext, "_drain_patched", False):
        return

    def _patched_drain(self, tick_clock, wait_clock):
        nop = self.nc.sync.nop()
        wait_clock.add_sem_waits(
            nop.ins, ScopedClock({None: tick_clock.global_clock})
        )
        si = nop.ins.sync_info
        waits = list(si.on_wait) if si and si.on_wait else []
        if len(waits) > 1:
            si.on_wait = waits[:1]
            for w in waits[1:]:
                extra = self.nc.sync.nop()
                extra.ins.sync_info = mybir.SyncInfo(on_wait=[w], on_update=[])
        self.nc.sync.drain()
        self.nc.all_engine_barrier()
        assert self.sems is not None
        popped = self.nc._tile_sem_poison_stack.pop()
        assert popped is self._sem_poison
        self.nc.clear_and_free_semaphores(list(self.sems.allocated().values()))
        self.nc.all_engine_barrier()

    tile.TileContext._drain_and_barrier = _patched_drain
    tile.TileContext._drain_patched = True


def _split_sync_waits(nc, max_waits=1):
    import concourse.mybir as mybir

    for f in nc.m.functions:
        for bb in f.blocks:
            new_insts = []
            dirty = False
            for inst in bb.instructions:
                si = inst.sync_info
                waits = list(si.on_wait) if si and si.on_wait else []
                if len(waits) > max_waits:
                    dirty = True
                    for i, w in enumerate(waits[max_waits:]):
                        nop = mybir.InstNoOp(
                            name=f"{inst.name}-wsplit{i}", ins=[], outs=[]
                        )
                        nop.engine = inst.engine
                        nop.sync_info = mybir.SyncInfo(on_wait=[w], on_update=[])
                        nc.register_instruction(nop)
                        new_insts.append(nop)
                    si.on_wait = waits[:max_waits]
                new_insts.append(inst)
            if dirty:
                bb.instructions = new_insts


# ---------------------------------------------------------------------------
# device kernel: one expert's gated FFN over C gathered tokens
# ---------------------------------------------------------------------------
def _build_nc(C, reps=1, do_dma=True, do_p1=True, do_p2=True, dma_in_loop=True):
    import concourse.bass as bass
    import concourse.mybir as mybir
    from concourse import tile
    from contextlib import nullcontext

    _install_tile_fixes()

    dt = mybir.dt
    f32 = dt.float32
    f32r = dt.float32r
    n_tb = C // TB
    n_tp = C // 128

    nc = bass.Bass("TRN2", target_bir_lowering=False, debug=False)
    xt = nc.dram_tensor("xt", [H, C], f32r, kind="ExternalInput").ap()
    wg = nc.dram_tensor("wg", [H, I], f32r, kind="ExternalInput").ap()
    wu = nc.dram_tensor("wu", [H, I], f32r, kind="ExternalInput").ap()
    wd = nc.dram_tensor("wd", [I, H], f32r, kind="ExternalInput").ap()
    wv = nc.dram_tensor("wv", [C, 1], f32, kind="ExternalInput").ap()
    y = nc.dram_tensor("y", [C, H], f32, kind="ExternalOutput").ap()

    with tile.TileContext(nc) as tc:
        with (
            tc.tile_pool(name="xw", bufs=1) as xwp,
            tc.tile_pool(name="inter", bufs=1) as interp,
            tc.tile_pool(name="tmp", bufs=4) as tmpp,
            tc.tile_pool(name="ysb", bufs=3) as ysbp,
            tc.tile_pool(name="ps", bufs=1, space="PSUM") as psp,
        ):
            xt_s = xwp.tile([128, 8, C], f32r, tag="xt")
            wg_s = xwp.tile([128, 8, I], f32r, tag="wg")
            wu_s = xwp.tile([128, 8, I], f32r, tag="wu")
            wd_s = xwp.tile([128, 8, H], f32r, tag="wd")
            wv_s = xwp.tile([128, n_tp], f32, tag="wv")
            inter_s = interp.tile([128, 8, C], f32r, tag="inter")

            if not do_p1 and do_p2:
                nc.vector.memset(inter_s[:].bitcast(f32), 0.0)

            def emit_dma():
                xt_r = xt.rearrange("(ht p) c -> p ht c", p=128)
                wg_r = wg.rearrange("(ht p) i -> p ht i", p=128)
                wu_r = wu.rearrange("(ht p) i -> p ht i", p=128)
                wd_r = wd.rearrange("(it p) h -> p it h", p=128)
                # interleave in phase-1 consumption order: i-tile 0 needs
                # xt[ht] + wg/wu[ht, ih0] for every ht before it can finish
                h0 = slice(0, 512)
                h1 = slice(512, 1024)
                for ht in range(8):
                    nc.sync.dma_start(out=xt_s[:, ht, :], in_=xt_r[:, ht, :])
                    nc.sync.dma_start(out=wg_s[:, ht, h0], in_=wg_r[:, ht, h0])
                    nc.sync.dma_start(out=wu_s[:, ht, h0], in_=wu_r[:, ht, h0])
                for ht in range(8):
                    nc.sync.dma_start(out=wg_s[:, ht, h1], in_=wg_r[:, ht, h1])
                    nc.sync.dma_start(out=wu_s[:, ht, h1], in_=wu_r[:, ht, h1])
                for ht in range(8):
                    nc.sync.dma_start(out=wd_s[:, ht, :], in_=wd_r[:, ht, :])
                nc.sync.dma_start(
                    out=wv_s[:], in_=wv.rearrange("(tp p) o -> p (tp o)", p=128)
                )

            if do_dma and not dma_in_loop:
                emit_dma()
            loop = tc.For_i(0, reps, 1) if reps > 1 else nullcontext()
            with loop:
                if do_dma and dma_in_loop:
                    emit_dma()

                # phase 1: g/u GEMMs ([I,C] = Wg/WuT @ XT), silu(g)*u -> inter
                for it in range(8 if do_p1 else 0):
                    pg = [
                        psp.tile([128, TB], f32, tag=f"g{tb}", name=f"pg{tb}")
                        for tb in range(n_tb)
                    ]
                    pu = [
                        psp.tile([128, TB], f32, tag=f"u{tb}", name=f"pu{tb}")
                        for tb in range(n_tb)
                    ]
                    isl = slice(it * 128, (it + 1) * 128)
                    for ht in range(8):
                        st, sp = ht == 0, ht == 7
                        for tb in range(n_tb):
                            tsl = slice(tb * TB, (tb + 1) * TB)
                            nc.tensor.matmul(
                                pg[tb][:], wg_s[:, ht, isl], xt_s[:, ht, tsl],
                                start=st, stop=sp,
                            )
                        for tb in range(n_tb):
                            tsl = slice(tb * TB, (tb + 1) * TB)
                            nc.tensor.matmul(
                                pu[tb][:], wu_s[:, ht, isl], xt_s[:, ht, tsl],
                                start=st, stop=sp,
                            )
                    for tb in range(n_tb):
                        tsl = slice(tb * TB, (tb + 1) * TB)
                        sil = tmpp.tile([128, TB], f32, tag="sil")
                        nc.scalar.activation(
                            sil[:], pg[tb][:], mybir.ActivationFunctionType.Silu
                        )
                        nc.vector.tensor_mul(
                            inter_s[:, it, tsl], sil[:], pu[tb][:]
                        )

                # phase 2: y[C,H] = interT @ Wd, scaled by the per-token gate
                for tp in range(n_tp if do_p2 else 0):
                    tsl = slice(tp * 128, (tp + 1) * 128)
                    py0 = psp.tile([128, 512], f32, tag="y0")
                    py1 = psp.tile([128, 512], f32, tag="y1")
                    for it in range(8):
                        st, sp = it == 0, it == 7
                        lhsT = inter_s[:, it, tsl]
                        nc.tensor.matmul(
                            py0[:], lhsT, wd_s[:, it, 0:512], start=st, stop=sp
                        )
                        nc.tensor.matmul(
                            py1[:], lhsT, wd_s[:, it, 512:1024], start=st, stop=sp
                        )
                    for hh, py in ((0, py0), (1, py1)):
                        y_sb = ysbp.tile([128, 512], f32, tag="ysb")
                        nc.vector.tensor_scalar_mul(
                            y_sb[:], py[:], wv_s[:, tp : tp + 1]
                        )
                        nc.sync.dma_start(
                            out=y[tsl, hh * 512 : (hh + 1) * 512], in_=y_sb[:]
                        )

    _split_sync_waits(nc)
    return nc


def _get_nc(C):
    if C not in _nc_cache:
        _nc_cache[C] = _build_nc(C)
    return _nc_cache[C]


# ---------------------------------------------------------------------------
# host: routing (fp32, mirrors the reference), dispatch, combine
# ---------------------------------------------------------------------------
def kernel(x, W_router, Wg, Wu, Wd):
    from concourse.bass_utils import run_bass_kernel_spmd

    x = np.asarray(x, dtype=np.float32)
    W_router = np.asarray(W_router, dtype=np.float32)
    Wg = np.asarray(Wg, dtype=np.float32)
    Wu = np.asarray(Wu, dtype=np.float32)
    Wd = np.asarray(Wd, dtype=np.float32)

    Bq, Sq, Hq = x.shape
    flat = np.ascontiguousarray(x.reshape(-1, Hq))
    T = flat.shape[0]

    # router: softmax + top-2 + renormalized combine weights (all fp32)
    logits = flat @ W_router
    m = logits.max(axis=-1, keepdims=True)
    ex = np.exp(logits - m)
    probs = (ex / ex.sum(axis=-1, keepdims=True)).astype(np.float32)
    ar = np.arange(T)
    idx1 = probs.argmax(axis=-1)
    p1 = probs[ar, idx1]
    probs_m = probs.copy()
    probs_m[ar, idx1] = -np.inf
    idx2 = probs_m.argmax(axis=-1)
    p2 = probs[ar, idx2]
    denom = np.maximum(p1 + p2, np.float32(1e-9))
    w1 = (p1 / denom).astype(np.float32)
    w2 = (p2 / denom).astype(np.float32)

    # aux load-balancing loss (fp32, as in the reference)
    routed = np.zeros((T, E), np.float32)
    np.add.at(routed, (ar, idx1), np.float32(1.0))
    np.add.at(routed, (ar, idx2), np.float32(1.0))
    aux_loss = np.float32(
        E * np.sum(routed.mean(axis=0) * probs.mean(axis=0)) * AUX_COEF
    )

    # dispatch: per-expert token lists, padded to a common capacity C
    pair_e = np.concatenate([idx1, idx2])
    pair_t = np.concatenate([ar, ar])
    pair_w = np.concatenate([w1, w2])
    order = np.argsort(pair_e, kind="stable")
    pair_e, pair_t, pair_w = pair_e[order], pair_t[order], pair_w[order]
    counts = np.bincount(pair_e, minlength=E)
    offsets = np.zeros(E + 1, np.int64)
    np.cumsum(counts, out=offsets[1:])
    C = max(TB, int(np.ceil(counts.max() / TB)) * TB)

    in_maps = []
    tok_of = []
    for e in range(E):
        te = pair_t[offsets[e] : offsets[e + 1]]
        we = pair_w[offsets[e] : offsets[e + 1]]
        n_e = len(te)
        xg = np.zeros((C, Hq), np.float32)
        xg[:n_e] = flat[te]
        wv = np.zeros((C, 1), np.float32)
        wv[:n_e, 0] = we
        in_maps.append(
            {
                "xt": np.ascontiguousarray(xg.T),
                "wg": np.ascontiguousarray(Wg[e]),
                "wu": np.ascontiguousarray(Wu[e]),
                "wd": np.ascontiguousarray(Wd[e]),
                "wv": wv,
            }
        )
        tok_of.append(te)

    nc = _get_nc(C)
    kernel.LAST = (nc, in_maps)
    res = run_bass_kernel_spmd(nc, in_maps, core_ids=list(range(E)))

    out = np.zeros((T, Hq), np.float32)
    for e in range(E):
        te = tok_of[e]
        out[te] += res.results[e]["y"][: len(te)]

    return out.reshape(Bq, Sq, Hq), aux_loss
